# revision 26
# baseline (speedup 1.0000x reference)
"""Swin-style windowed multi-head attention on 8 Trainium2 NeuronCores.

Problem: nn_Attention_86792699118108
  x [16, 3136, 768], 56x56 spatial, window 14x14 (no padding needed),
  12 heads, head_dim 64. 256 independent windows -> 32 windows per core.

Strategy (data-parallel over windows, v2 — dense-PE redesign):
  host: window-partition x -> bf16; pre-transpose/scale weights; permute
        heads by SIGMA so device-side attention slots are contiguous;
        pre-gather + exp the relative-position bias (exp(s+b)=exp(s)exp(b));
        fold v_bias/proj_b into a host-side output bias (exact).
  device (per core, SPMD), per group of 4 windows:
    xT (chan-major) via DMA-transpose ->
    q/k GEMM (psum 1 bank / chunk, ACT-copy eviction) ;
    v GEMM (token-major, interleaved 65-col layout with ones column) ->
    per window, per 4-head granule:
      scores: 4 matmuls into a 2-bank psum tile; head pairs at row
      groups (0,64) x banks (A,B) co-issue concurrently (PE row tiling);
      exp (ACT, one 4-head op) -> *exp(rpb) (DVE) ->
    per head-pair: AV with ones row (softmax sums free) into 1 bank;
      reciprocal (DVE, direct from psum), gpsimd partition-broadcast,
      normalize-on-eviction (DVE) ->
    proj GEMM (streams wp) -> ACT eviction -> DMA y.
    Next group's q/k/v GEMM thunks are interleaved as PE filler so the
    tensor engine never idles (keeps the HAM clock-gate at 2.4 GHz).
  host: window-reverse, + (v_bias @ proj_w.T + proj_b).
"""

import numpy as np
import ml_dtypes

WS = 14
NH = 12
HD = 64
C = 768
N = WS * WS  # 196 tokens per window
NCORES = 8

# slot -> original head; chosen so that the scores psum blocks
# [bankA0, bankA1, bankB0, bankB1] = heads [4g, 4g+2, 4g+1, 4g+3] read out
# in contiguous slot order (head pairs (2i,2i+1) co-issue into banks A/B).
SIGMA = [0, 2, 1, 3, 4, 6, 5, 7, 8, 10, 9, 11]

_BF16 = ml_dtypes.bfloat16

# debug/bisect flags (affect program structure; not part of cache key --
# set before first kernel() call only)
FLAG_EXP3D = False      # exp via two <=3D-AP ops instead of one 4D op
# reciprocal_approx_fast (custom DVE ucode) reading PSUM directly faults on
# real HW (sim accepts it) — stage the sums row through SBUF via ACT first.
FLAG_RECIP_SBUF = True
FLAG_RREP128 = False    # partition_broadcast to 128 rows (baseline shape)
FLAG_AV2BANK = False    # AV psum [65,1024], j-stride 512 (baseline layout)

_prog_cache = {}


def _rel_index(ws):
    coords = np.stack(np.meshgrid(np.arange(ws), np.arange(ws), indexing="ij"))
    cf = coords.reshape(2, -1)
    rel = (cf[:, :, None] - cf[:, None, :]).transpose(1, 2, 0).astype(np.int64)
    rel[..., 0] += ws - 1
    rel[..., 1] += ws - 1
    rel[..., 0] *= 2 * ws - 1
    return rel.sum(-1)


def _build_program(n_win, has_qbias):
    import concourse.mybir as mybir
    import concourse.tile as tile
    from concourse import bacc
    from contextlib import ExitStack

    assert n_win % 4 == 0
    n_grp = n_win // 4
    n_tok = n_win * N

    BF = mybir.dt.bfloat16
    F32 = mybir.dt.float32
    AF = mybir.ActivationFunctionType

    MC = [(0, 128), (128, 68)]  # key/token chunks within a 196-token window

    nc = bacc.Bacc("TRN2", target_bir_lowering=False, debug=False,
                   num_devices=NCORES)

    x = nc.dram_tensor("x", [n_tok, C], BF, kind="ExternalInput")
    wqkvT = nc.dram_tensor("wqkvT", [C, 3 * C], BF, kind="ExternalInput")
    wpT = nc.dram_tensor("wpT", [C, C], BF, kind="ExternalInput")
    er = nc.dram_tensor("er", [N, NH * N], BF, kind="ExternalInput")
    if has_qbias:
        qb = nc.dram_tensor("qb", [128, 6], BF, kind="ExternalInput")
    y = nc.dram_tensor("y", [n_tok, C], F32, kind="ExternalOutput")

    with ExitStack() as ctx:
        tc = ctx.enter_context(tile.TileContext(nc))
        consts = ctx.enter_context(tc.tile_pool(name="consts", bufs=1))
        grp = ctx.enter_context(tc.tile_pool(name="grp", bufs=2))
        win = ctx.enter_context(tc.tile_pool(name="win", bufs=2))
        # PSUM budget: 8 banks total.
        #   scp: scores, [*,1024] f32 = 2 banks per slot, 2 slots  -> 4
        #   avp: AV,     [65,392] f32 = 1 bank per slot, 2 slots   -> 2
        #   mmp: qkv-thunk + proj psum, 1 bank per slot, 2 slots   -> 2
        scp = ctx.enter_context(tc.tile_pool(
            name="scp", bufs=1 if FLAG_AV2BANK else 2, space="PSUM"))
        avp = ctx.enter_context(tc.tile_pool(name="avp", bufs=2, space="PSUM"))
        mmp = ctx.enter_context(tc.tile_pool(name="mmp", bufs=2, space="PSUM"))

        def emit_xT(g):
            t0 = g * 4 * N
            xT = []
            for ic in range(6):
                t = grp.tile([128, 4 * N], BF, tag=f"xT{ic}", name=f"xT{ic}")
                nc.sync.dma_start(
                    out=t,
                    in_=x[t0:t0 + 4 * N, ic * 128:(ic + 1) * 128],
                    transpose=True)
                xT.append(t)
            return xT

        # ---- constants (spread across DMA queues; wp/er deprioritized —
        # they are first needed ~20-40us in, wq/xT gate the first matmul) --
        def emit_wq(ic, eng):
            t = consts.tile([128, 3 * C], BF, tag=f"wq{ic}", name=f"wq{ic}")
            eng.dma_start(out=t, in_=wqkvT[ic * 128:(ic + 1) * 128, :])
            return t

        # startup critical path: the first q/k thunk consumes (wq[ic], xT[ic])
        # incrementally, so land wq0 and the xT transposes first on sync and
        # stripe the rest across the scalar queue.
        wq = [None] * 6
        wq[0] = emit_wq(0, nc.sync)
        xT0 = emit_xT(0)
        for ic in range(1, 6):
            wq[ic] = emit_wq(ic, nc.scalar if ic % 2 else nc.sync)
        er_t = []
        for mci, (mo, msz) in enumerate(MC):
            t = consts.tile([msz, NH * N], BF, tag=f"er{mci}", name=f"er{mci}")
            nc.scalar.dma_start(out=t, in_=er[mo:mo + msz, :])
            er_t.append(t)
        wp = []
        for ic in range(6):
            t = consts.tile([128, C], BF, tag=f"wp{ic}", name=f"wp{ic}")
            nc.scalar.dma_start(out=t, in_=wpT[ic * 128:(ic + 1) * 128, :])
            wp.append(t)
        if has_qbias:
            qb_t = consts.tile([128, 6], BF, tag="qb", name="qb_t")
            nc.sync.dma_start(out=qb_t, in_=qb[:, :])

        def load_thunks(g, xT):
            """q/k/v GEMM for group g as fine-grained thunks (~1us PE each),
            interleaved into the previous group's attention as PE filler."""
            qk = [grp.tile([128, 4 * N], BF, tag=f"qk{oc}", name=f"qk{oc}")
                  for oc in range(12)]
            v_t = {}
            for w4 in range(4):
                for mci, (mo, msz) in enumerate(MC):
                    v_t[(w4, mci)] = grp.tile(
                        [128, NH * 65], BF, tag=f"v{w4}_{mci}",
                        name=f"v{w4}_{mci}")
            thunks = []

            def mk_qk(oc, s):
                def f():
                    ps = mmp.tile([128, 392], F32, tag="mm", name="psqk")
                    for ic in range(6):
                        nc.tensor.matmul(
                            ps,
                            wq[ic][:, oc * 128:(oc + 1) * 128],
                            xT[ic][:, s * 392:(s + 1) * 392],
                            start=(ic == 0), stop=(ic == 5))
                    nc.scalar.copy(qk[oc][:, s * 392:(s + 1) * 392], ps)
                return f

            def mk_v(w4, mci, half):
                mo, msz = MC[mci]

                def f():
                    vt = v_t[(w4, mci)]
                    vr = vt.rearrange("p (h e) -> p h e", e=65)
                    ps = mmp.tile([128, 384], F32, tag="mm", name="psv")
                    for ic in range(6):
                        nc.tensor.matmul(
                            ps[:msz],
                            xT[ic][:, w4 * N + mo: w4 * N + mo + msz],
                            wq[ic][:, 1536 + half * 384: 1536 + (half + 1) * 384],
                            start=(ic == 0), stop=(ic == 5))
                    nc.vector.tensor_copy(
                        vr[:msz, half * 6:(half + 1) * 6, 0:64],
                        ps[:msz].rearrange("p (h e) -> p h e", e=64))
                    if half == 0:
                        nc.vector.memset(vr[:msz, :, 64:65], 1.0)
                return f

            for oc in range(12):
                for s in range(2):
                    thunks.append((g, -1, mk_qk(oc, s)))
            for w4 in range(4):
                for mci in range(2):
                    for half in range(2):
                        thunks.append((g, w4, mk_v(w4, mci, half)))
            return qk, v_t, thunks

        def emit_scores(g, w4, qk, filler):
            w0 = w4 * N
            ex = []
            at = []
            for mci, (mo, msz) in enumerate(MC):
                ex.append(win.tile([msz, NH * N], BF, tag=f"ex{mci}",
                                   name=f"ex{mci}"))
                at.append(win.tile([msz, NH * N], BF, tag=f"at{mci}",
                                   name=f"at{mci}"))

            # -- scores + exp + rel-bias multiply, per 4-head granule -----
            for gr in range(3):
                for mci, (mo, msz) in enumerate(MC):
                    ps = scp.tile([128, 1024], F32, tag="sc", name="pssc")
                    # psum blocks: A0=0, A1=196 (bank A); B0=512, B1=708
                    # (bank B).  Head pairs of one qk tile go to different
                    # banks + different PE row groups -> co-issue.
                    kta, ktb = qk[6 + 2 * gr], qk[6 + 2 * gr + 1]
                    qta, qtb = qk[2 * gr], qk[2 * gr + 1]
                    nc.tensor.matmul(  # head 4gr -> A0
                        ps[:msz, 0:N],
                        kta[0:64, w0 + mo: w0 + mo + msz],
                        qta[0:64, w0:w0 + N],
                        start=True, stop=False)
                    nc.tensor.matmul(  # head 4gr+1 -> B0
                        ps[:msz, 512:512 + N],
                        kta[64:128, w0 + mo: w0 + mo + msz],
                        qta[64:128, w0:w0 + N],
                        start=True, stop=False)
                    if has_qbias:
                        # delta(m) = q_bias . k_m per head, psum cols 904+k
                        # (bank B group), exp bias operands.
                        dk = [(kta, 0, 2 * gr), (ktb, 0, 2 * gr + 1),
                              (kta, 64, 2 * gr), (ktb, 64, 2 * gr + 1)]
                        for k, (kt, ro, ti) in enumerate(dk):
                            nc.tensor.matmul(
                                ps[:msz, 904 + k:905 + k],
                                kt[ro:ro + 64, w0 + mo: w0 + mo + msz],
                                qb_t[ro:ro + 64, ti:ti + 1],
                                start=False, stop=False)
                    nc.tensor.matmul(  # head 4gr+2 -> A1
                        ps[:msz, N:2 * N],
                        ktb[0:64, w0 + mo: w0 + mo + msz],
                        qtb[0:64, w0:w0 + N],
                        start=False, stop=True)
                    nc.tensor.matmul(  # head 4gr+3 -> B1
                        ps[:msz, 512 + N:512 + 2 * N],
                        ktb[64:128, w0 + mo: w0 + mo + msz],
                        qtb[64:128, w0:w0 + N],
                        start=False, stop=True)

                    exs = (ex[mci].rearrange("p (s n) -> p s n", n=N)
                           [:, 4 * gr:4 * gr + 4, :])
                    if has_qbias:
                        for k in range(4):
                            pcol = (k // 2) * 512 + (k % 2) * N
                            nc.scalar.activation(
                                exs[:, k, :], ps[:msz, pcol:pcol + N],
                                AF.Exp, bias=ps[:msz, 904 + k:905 + k])
                    elif FLAG_EXP3D:
                        for b in range(2):
                            nc.scalar.activation(
                                exs[:, 2 * b:2 * b + 2, :],
                                ps[:msz, 512 * b:512 * b + 2 * N]
                                    .rearrange("p (k n) -> p k n", n=N),
                                AF.Exp)
                    else:
                        nc.scalar.activation(
                            exs.rearrange("p (b k) n -> p b k n", b=2),
                            ps[:msz].rearrange("p (b c) -> p b c", b=2)
                                [:, :, 0:2 * N]
                                .rearrange("p b (k n) -> p b k n", n=N),
                            AF.Exp)
                    nc.vector.tensor_mul(
                        at[mci][:, 4 * gr * N:(4 * gr + 4) * N],
                        ex[mci][:, 4 * gr * N:(4 * gr + 4) * N],
                        er_t[mci][:, 4 * gr * N:(4 * gr + 4) * N])
                filler(1)
            return at

        def emit_av_proj(g, w4, at, v_t, filler):
            t0 = g * 4 * N
            w0 = w4 * N
            # -- AV + softmax-normalize, per head pair --------------------
            aoT = [win.tile([128, N], BF, tag=f"aoT{i}", name=f"aoT{i}")
                   for i in range(6)]
            jst = 512 if FLAG_AV2BANK else N  # psum column stride per head
            for p6 in range(6):
                if FLAG_AV2BANK:
                    ps = avp.tile([65, 1024], F32, tag="av", name="psav")
                else:
                    ps = avp.tile([65, 392], F32, tag="av", name="psav")
                for j in range(2):
                    s = 2 * p6 + j
                    for mci, (mo, msz) in enumerate(MC):
                        nc.tensor.matmul(
                            ps[:, j * jst:j * jst + N],
                            v_t[(w4, mci)][:msz, s * 65:(s + 1) * 65],
                            at[mci][:, s * N:(s + 1) * N],
                            start=(mci == 0) if FLAG_AV2BANK
                            else (j == 0 and mci == 0),
                            stop=(mci == 1) if FLAG_AV2BANK
                            else (j == 1 and mci == 1))
                sums_ap = (ps[64:65, :].rearrange("p (j c) -> p j c", j=2)
                           [:, :, 0:N] if FLAG_AV2BANK else ps[64:65, :])
                if FLAG_RECIP_SBUF:
                    sm = win.tile([1, 2 * N], F32, tag="sm", name="sm")
                    nc.scalar.activation(
                        sm.rearrange("p (j n) -> p j n", n=N)
                        if FLAG_AV2BANK else sm,
                        sums_ap, AF.Copy)
                    rsrc = sm
                else:
                    rsrc = sums_ap
                rr = win.tile([1, 2 * N], F32, tag="rr", name="rr")
                nc.vector.reciprocal_approx_fast(
                    rr.rearrange("p (j n) -> p j n", n=N)
                    if (FLAG_AV2BANK and not FLAG_RECIP_SBUF) else rr, rsrc)
                nrep = 128 if FLAG_RREP128 else 64
                rrep = win.tile([nrep, 2 * N], F32, tag="rrep", name="rrep")
                nc.gpsimd.partition_broadcast(rrep, rr)
                for j in range(2):
                    nc.vector.tensor_mul(
                        aoT[p6][j * 64:(j + 1) * 64, :],
                        ps[0:64, j * jst:j * jst + N],
                        rrep[0:64, j * N:(j + 1) * N])
                if p6 % 2 == 1:
                    filler(1)

            # -- projection ----------------------------------------------
            filler(2)  # cover the p6=5 normalize latency before proj needs it
            ysb = [win.tile([128, C], F32, tag=f"ysb{i}", name=f"ysb{i}")
                   for i in range(2)]
            for mci, (mo, msz) in enumerate(MC):
                pss = [mmp.tile([128, 384], F32, tag="mm", name="psp")
                       for _ in range(2)]
                for ic in range(6):
                    for half in range(2):
                        nc.tensor.matmul(
                            pss[half][:msz],
                            aoT[ic][:, mo:mo + msz],
                            wp[ic][:, half * 384:(half + 1) * 384],
                            start=(ic == 0), stop=(ic == 5))
                for half in range(2):
                    nc.scalar.copy(
                        ysb[mci][:msz, half * 384:(half + 1) * 384],
                        pss[half][:msz])
                nc.sync.dma_start(
                    out=y[t0 + w0 + mo: t0 + w0 + mo + msz, :],
                    in_=ysb[mci][:msz, :])
                filler(1)

        # Software pipeline: group g+1's q/k GEMM thunks drain as PE filler
        # inside group g's windows; v GEMM thunks of each group may spill
        # into that group's own early windows (guarded by drain_upto), which
        # also gives the last group filler work against its softmax latency.
        from collections import deque
        thunk_q = deque()
        cur_g = [0]

        def filler(k):
            # v thunks of group g+1 are held back so they remain available
            # as filler inside g+1's own windows (its qk tiles must be ready
            # before g+1's scores anyway, but v(w) is only needed by AV(w))
            for _ in range(k):
                if thunk_q:
                    tg, tw, fn = thunk_q[0]
                    if tg > cur_g[0] and tw >= 0:
                        return
                    thunk_q.popleft()
                    fn()

        def drain_upto(g, w4):
            # everything this group's window w4 needs: all earlier groups'
            # thunks, group g's qk thunks, and its v thunks for w' <= w4
            while thunk_q:
                tg, tw, fn = thunk_q[0]
                if tg < g or (tg == g and (tw < 0 or tw <= w4)):
                    thunk_q.popleft()
                    fn()
                else:
                    break

        xT0 = emit_xT(0)
        qk_c, vt_c, th0 = load_thunks(0, xT0)
        for _, _, th in th0:
            th()
        for g in range(n_grp):
            if g + 1 < n_grp:
                xTn = emit_xT(g + 1)
                qk_n, vt_n, thunks = load_thunks(g + 1, xTn)
                thunk_q.extend(thunks)
            else:
                qk_n, vt_n = None, None
            for w4 in range(4):
                drain_upto(g, w4)
                at = emit_scores(g, w4, qk_c, filler)
                emit_av_proj(g, w4, at, vt_c, filler)
            qk_c, vt_c = qk_n, vt_n
            cur_g[0] = g + 1
        while thunk_q:
            thunk_q.popleft()[2]()

    nc.compile()
    return nc


def _get_program(n_win, has_qbias):
    key = (n_win, has_qbias)
    if key not in _prog_cache:
        _prog_cache[key] = _build_program(n_win, has_qbias)
    return _prog_cache[key]


def _host_prep(x, qkv_w, q_bias, v_bias, rel_bias_table, proj_w, proj_b, H, W):
    B = x.shape[0]
    nws = H // WS  # windows per side
    xw = (np.asarray(x, np.float32)
          .reshape(B, nws, WS, nws, WS, C)
          .transpose(0, 1, 3, 2, 4, 5)
          .reshape(-1, N, C))  # [Bw, 196, C]

    scale = HD ** -0.5
    wq_s = np.array(qkv_w, np.float32, copy=True)
    wq_s[0:C] *= scale
    wqkvT = np.ascontiguousarray(wq_s.T)  # [C, 3C] f32
    # permute the v output channels into SIGMA slot order
    wqkvT[:, 2 * C:] = (wqkvT[:, 2 * C:]
                        .reshape(C, NH, HD)[:, SIGMA, :].reshape(C, C))
    wqkvT = wqkvT.astype(_BF16)

    # wp rows (attention-concat input channels) into SIGMA slot order
    wpT = np.ascontiguousarray(np.asarray(proj_w, np.float32).T)  # [C_in, C_out]
    wpT = np.ascontiguousarray(
        wpT.reshape(NH, HD, C)[SIGMA].reshape(C, C)).astype(_BF16)

    idx = _rel_index(WS).reshape(-1)
    rpb = np.asarray(rel_bias_table, np.float32)[idx].reshape(N, N, NH)  # [n,m,h]
    er_arr = np.exp(rpb).transpose(1, 2, 0)  # [m, h, n]
    er_arr = er_arr[:, SIGMA, :]             # [m, slot, n]
    er = np.ascontiguousarray(er_arr.reshape(N, NH * N)).astype(_BF16)

    qbs = np.asarray(q_bias, np.float32) * scale
    has_qbias = bool(np.any(qbs))
    qb = np.ascontiguousarray(qbs.reshape(6, 128).T).astype(_BF16)  # [128, 6]

    # v_bias and proj_b folded into a host-side output bias (exact):
    # y = (A v_raw) Wp^T + (v_bias Wp^T + proj_b)
    hb = (np.asarray(v_bias, np.float32) @ np.asarray(proj_w, np.float32).T
          + np.asarray(proj_b, np.float32))

    xbf = np.ascontiguousarray(xw.reshape(-1, C)).astype(_BF16)
    return xbf, wqkvT, wpT, er, qb, has_qbias, hb


def kernel(x, qkv_w, q_bias, v_bias, rel_bias_table, proj_w, proj_b, H, W,
           _return_results=False):
    from concourse.bass_utils import run_bass_kernel_spmd

    x = np.asarray(x)
    B = x.shape[0]
    H = int(H)
    W = int(W)
    nws = H // WS

    xbf, wqkvT, wpT, er, qb, has_qbias, hb = _host_prep(
        x, qkv_w, q_bias, v_bias, rel_bias_table, proj_w, proj_b, H, W)

    Bw = B * nws * nws
    n_win_core = Bw // NCORES
    nc = _get_program(n_win_core, has_qbias)

    tok_core = n_win_core * N
    in_maps = []
    for c in range(NCORES):
        m = {
            "x": xbf[c * tok_core:(c + 1) * tok_core],
            "wqkvT": wqkvT, "wpT": wpT, "er": er,
        }
        if has_qbias:
            m["qb"] = qb
        in_maps.append(m)

    res = run_bass_kernel_spmd(nc, in_maps, list(range(NCORES)))
    yw = np.concatenate([res.results[c]["y"] for c in range(NCORES)], axis=0)
    out = (yw.reshape(B, nws, nws, WS, WS, C)
           .transpose(0, 1, 3, 2, 4, 5)
           .reshape(B, H * W, C).astype(np.float32))
    out += hb[None, None, :]
    if _return_results:
        return out, res
    return out


# revision 29
# speedup vs baseline: 1.0200x; 1.0200x over previous
"""Swin-style windowed multi-head attention on 8 Trainium2 NeuronCores.

Problem: nn_Attention_86792699118108
  x [16, 3136, 768], 56x56 spatial, window 14x14 (no padding needed),
  12 heads, head_dim 64. 256 independent windows -> 32 windows per core.

Strategy (data-parallel over windows, v2 — dense-PE redesign):
  host: window-partition x -> bf16; pre-transpose/scale weights; permute
        heads by SIGMA so device-side attention slots are contiguous;
        pre-gather + exp the relative-position bias (exp(s+b)=exp(s)exp(b));
        fold v_bias/proj_b into a host-side output bias (exact).
  device (per core, SPMD), per group of 4 windows:
    xT (chan-major) via DMA-transpose ->
    q/k GEMM (psum 1 bank / chunk, ACT-copy eviction) ;
    v GEMM (token-major, interleaved 65-col layout with ones column) ->
    per window, per 4-head granule:
      scores: 4 matmuls into a 2-bank psum tile; head pairs at row
      groups (0,64) x banks (A,B) co-issue concurrently (PE row tiling);
      exp (ACT, one 4-head op) -> *exp(rpb) (DVE) ->
    per head-pair: AV with ones row (softmax sums free) into 1 bank;
      reciprocal (DVE, direct from psum), gpsimd partition-broadcast,
      normalize-on-eviction (DVE) ->
    proj GEMM (streams wp) -> ACT eviction -> DMA y.
    Next group's q/k/v GEMM thunks are interleaved as PE filler so the
    tensor engine never idles (keeps the HAM clock-gate at 2.4 GHz).
  host: window-reverse, + (v_bias @ proj_w.T + proj_b).
"""

import numpy as np
import ml_dtypes

WS = 14
NH = 12
HD = 64
C = 768
N = WS * WS  # 196 tokens per window
NCORES = 8

# slot -> original head; chosen so that the scores psum blocks
# [bankA0, bankA1, bankB0, bankB1] = heads [4g, 4g+2, 4g+1, 4g+3] read out
# in contiguous slot order (head pairs (2i,2i+1) co-issue into banks A/B).
SIGMA = [0, 2, 1, 3, 4, 6, 5, 7, 8, 10, 9, 11]

_BF16 = ml_dtypes.bfloat16

# debug/bisect flags (affect program structure; not part of cache key --
# set before first kernel() call only)
FLAG_EXP3D = False      # exp via two <=3D-AP ops instead of one 4D op
# reciprocal_approx_fast (custom DVE ucode) reading PSUM directly faults on
# real HW (sim accepts it) — stage the sums row through SBUF via ACT first.
FLAG_RECIP_SBUF = True
FLAG_RREP128 = False    # partition_broadcast to 128 rows (baseline shape)
FLAG_AV2BANK = False    # AV psum [65,1024], j-stride 512 (baseline layout)
FLAG_QKEVIC_GPSIMD = False  # qk psum->sbuf eviction on gpsimd instead of ACT

_prog_cache = {}


def _rel_index(ws):
    coords = np.stack(np.meshgrid(np.arange(ws), np.arange(ws), indexing="ij"))
    cf = coords.reshape(2, -1)
    rel = (cf[:, :, None] - cf[:, None, :]).transpose(1, 2, 0).astype(np.int64)
    rel[..., 0] += ws - 1
    rel[..., 1] += ws - 1
    rel[..., 0] *= 2 * ws - 1
    return rel.sum(-1)


def _build_program(n_win, has_qbias):
    import concourse.mybir as mybir
    import concourse.tile as tile
    from concourse import bacc
    from contextlib import ExitStack

    assert n_win % 4 == 0
    n_grp = n_win // 4
    n_tok = n_win * N

    BF = mybir.dt.bfloat16
    F32 = mybir.dt.float32
    AF = mybir.ActivationFunctionType

    MC = [(0, 128), (128, 68)]  # key/token chunks within a 196-token window

    nc = bacc.Bacc("TRN2", target_bir_lowering=False, debug=False,
                   num_devices=NCORES)

    x = nc.dram_tensor("x", [n_tok, C], BF, kind="ExternalInput")
    wqkvT = nc.dram_tensor("wqkvT", [C, 3 * C], BF, kind="ExternalInput")
    wpT = nc.dram_tensor("wpT", [C, C], BF, kind="ExternalInput")
    er = nc.dram_tensor("er", [N, NH * N], BF, kind="ExternalInput")
    if has_qbias:
        qb = nc.dram_tensor("qb", [128, 6], BF, kind="ExternalInput")
    y = nc.dram_tensor("y", [n_tok, C], F32, kind="ExternalOutput")

    with ExitStack() as ctx:
        tc = ctx.enter_context(tile.TileContext(nc))
        consts = ctx.enter_context(tc.tile_pool(name="consts", bufs=1))
        grp = ctx.enter_context(tc.tile_pool(name="grp", bufs=2))
        win = ctx.enter_context(tc.tile_pool(name="win", bufs=2))
        # PSUM budget: 8 banks total.
        #   scp: scores, [*,1024] f32 = 2 banks per slot, 2 slots  -> 4
        #   avp: AV,     [65,392] f32 = 1 bank per slot, 2 slots   -> 2
        #   mmp: qkv-thunk + proj psum, 1 bank per slot, 2 slots   -> 2
        scp = ctx.enter_context(tc.tile_pool(
            name="scp", bufs=1 if FLAG_AV2BANK else 2, space="PSUM"))
        avp = ctx.enter_context(tc.tile_pool(name="avp", bufs=2, space="PSUM"))
        mmp = ctx.enter_context(tc.tile_pool(name="mmp", bufs=2, space="PSUM"))

        def emit_xT(g):
            t0 = g * 4 * N
            xT = []
            for ic in range(6):
                t = grp.tile([128, 4 * N], BF, tag=f"xT{ic}", name=f"xT{ic}")
                nc.sync.dma_start(
                    out=t,
                    in_=x[t0:t0 + 4 * N, ic * 128:(ic + 1) * 128],
                    transpose=True)
                xT.append(t)
            return xT

        # ---- constants (spread across DMA queues; wp/er deprioritized —
        # they are first needed ~20-40us in, wq/xT gate the first matmul) --
        def emit_wq(ic, eng):
            t = consts.tile([128, 3 * C], BF, tag=f"wq{ic}", name=f"wq{ic}")
            eng.dma_start(out=t, in_=wqkvT[ic * 128:(ic + 1) * 128, :])
            return t

        # startup critical path: the first q/k thunk consumes (wq[ic], xT[ic])
        # incrementally, so land wq0 and the xT transposes first on sync and
        # stripe the rest across the scalar queue.
        wq = [None] * 6
        wq[0] = emit_wq(0, nc.sync)
        xT0 = emit_xT(0)
        for ic in range(1, 6):
            wq[ic] = emit_wq(ic, nc.scalar if ic % 2 else nc.sync)
        er_t = []
        for mci, (mo, msz) in enumerate(MC):
            t = consts.tile([msz, NH * N], BF, tag=f"er{mci}", name=f"er{mci}")
            nc.scalar.dma_start(out=t, in_=er[mo:mo + msz, :])
            er_t.append(t)
        wp = []
        for ic in range(6):
            t = consts.tile([128, C], BF, tag=f"wp{ic}", name=f"wp{ic}")
            nc.scalar.dma_start(out=t, in_=wpT[ic * 128:(ic + 1) * 128, :])
            wp.append(t)
        if has_qbias:
            qb_t = consts.tile([128, 6], BF, tag="qb", name="qb_t")
            nc.sync.dma_start(out=qb_t, in_=qb[:, :])

        def load_thunks(g, xT):
            """q/k/v GEMM for group g as fine-grained thunks (~1us PE each),
            interleaved into the previous group's attention as PE filler."""
            qk = [grp.tile([128, 4 * N], BF, tag=f"qk{oc}", name=f"qk{oc}")
                  for oc in range(12)]
            v_t = {}
            for w4 in range(4):
                for mci, (mo, msz) in enumerate(MC):
                    v_t[(w4, mci)] = grp.tile(
                        [128, NH * 65], BF, tag=f"v{w4}_{mci}",
                        name=f"v{w4}_{mci}")
            thunks = []

            def mk_qk(oc, s):
                def f():
                    ps = mmp.tile([128, 392], F32, tag="mm", name="psqk")
                    for ic in range(6):
                        nc.tensor.matmul(
                            ps,
                            wq[ic][:, oc * 128:(oc + 1) * 128],
                            xT[ic][:, s * 392:(s + 1) * 392],
                            start=(ic == 0), stop=(ic == 5))
                    if FLAG_QKEVIC_GPSIMD:
                        nc.gpsimd.tensor_copy(
                            qk[oc][:, s * 392:(s + 1) * 392], ps)
                    else:
                        nc.scalar.copy(qk[oc][:, s * 392:(s + 1) * 392], ps)
                return f

            def mk_v(w4, mci, half):
                mo, msz = MC[mci]

                def f():
                    vt = v_t[(w4, mci)]
                    vr = vt.rearrange("p (h e) -> p h e", e=65)
                    ps = mmp.tile([128, 384], F32, tag="mm", name="psv")
                    for ic in range(6):
                        nc.tensor.matmul(
                            ps[:msz],
                            xT[ic][:, w4 * N + mo: w4 * N + mo + msz],
                            wq[ic][:, 1536 + half * 384: 1536 + (half + 1) * 384],
                            start=(ic == 0), stop=(ic == 5))
                    nc.vector.tensor_copy(
                        vr[:msz, half * 6:(half + 1) * 6, 0:64],
                        ps[:msz].rearrange("p (h e) -> p h e", e=64))
                    if half == 0:
                        nc.vector.memset(vr[:msz, :, 64:65], 1.0)
                return f

            for oc in range(12):
                for s in range(2):
                    thunks.append((g, -1, mk_qk(oc, s)))
            for w4 in range(4):
                for mci in range(2):
                    for half in range(2):
                        thunks.append((g, w4, mk_v(w4, mci, half)))
            return qk, v_t, thunks

        def emit_scores(g, w4, qk, filler):
            w0 = w4 * N
            ex = []
            at = []
            for mci, (mo, msz) in enumerate(MC):
                ex.append(win.tile([msz, NH * N], BF, tag=f"ex{mci}",
                                   name=f"ex{mci}"))
                at.append(win.tile([msz, NH * N], BF, tag=f"at{mci}",
                                   name=f"at{mci}"))

            # -- scores + exp + rel-bias multiply, per 4-head granule -----
            for gr in range(3):
                for mci, (mo, msz) in enumerate(MC):
                    ps = scp.tile([128, 1024], F32, tag="sc", name="pssc")
                    # psum blocks: A0=0, A1=196 (bank A); B0=512, B1=708
                    # (bank B).  Head pairs of one qk tile go to different
                    # banks + different PE row groups -> co-issue.
                    kta, ktb = qk[6 + 2 * gr], qk[6 + 2 * gr + 1]
                    qta, qtb = qk[2 * gr], qk[2 * gr + 1]
                    nc.tensor.matmul(  # head 4gr -> A0
                        ps[:msz, 0:N],
                        kta[0:64, w0 + mo: w0 + mo + msz],
                        qta[0:64, w0:w0 + N],
                        start=True, stop=False)
                    nc.tensor.matmul(  # head 4gr+1 -> B0
                        ps[:msz, 512:512 + N],
                        kta[64:128, w0 + mo: w0 + mo + msz],
                        qta[64:128, w0:w0 + N],
                        start=True, stop=False)
                    if has_qbias:
                        # delta(m) = q_bias . k_m per head, psum cols 904+k
                        # (bank B group), exp bias operands.
                        dk = [(kta, 0, 2 * gr), (ktb, 0, 2 * gr + 1),
                              (kta, 64, 2 * gr), (ktb, 64, 2 * gr + 1)]
                        for k, (kt, ro, ti) in enumerate(dk):
                            nc.tensor.matmul(
                                ps[:msz, 904 + k:905 + k],
                                kt[ro:ro + 64, w0 + mo: w0 + mo + msz],
                                qb_t[ro:ro + 64, ti:ti + 1],
                                start=False, stop=False)
                    nc.tensor.matmul(  # head 4gr+2 -> A1
                        ps[:msz, N:2 * N],
                        ktb[0:64, w0 + mo: w0 + mo + msz],
                        qtb[0:64, w0:w0 + N],
                        start=False, stop=True)
                    nc.tensor.matmul(  # head 4gr+3 -> B1
                        ps[:msz, 512 + N:512 + 2 * N],
                        ktb[64:128, w0 + mo: w0 + mo + msz],
                        qtb[64:128, w0:w0 + N],
                        start=False, stop=True)

                    exs = (ex[mci].rearrange("p (s n) -> p s n", n=N)
                           [:, 4 * gr:4 * gr + 4, :])
                    if has_qbias:
                        for k in range(4):
                            pcol = (k // 2) * 512 + (k % 2) * N
                            nc.scalar.activation(
                                exs[:, k, :], ps[:msz, pcol:pcol + N],
                                AF.Exp, bias=ps[:msz, 904 + k:905 + k])
                    elif FLAG_EXP3D:
                        for b in range(2):
                            nc.scalar.activation(
                                exs[:, 2 * b:2 * b + 2, :],
                                ps[:msz, 512 * b:512 * b + 2 * N]
                                    .rearrange("p (k n) -> p k n", n=N),
                                AF.Exp)
                    else:
                        nc.scalar.activation(
                            exs.rearrange("p (b k) n -> p b k n", b=2),
                            ps[:msz].rearrange("p (b c) -> p b c", b=2)
                                [:, :, 0:2 * N]
                                .rearrange("p b (k n) -> p b k n", n=N),
                            AF.Exp)
                    nc.vector.tensor_mul(
                        at[mci][:, 4 * gr * N:(4 * gr + 4) * N],
                        ex[mci][:, 4 * gr * N:(4 * gr + 4) * N],
                        er_t[mci][:, 4 * gr * N:(4 * gr + 4) * N])
                filler(1)
            return at

        def emit_av_proj(g, w4, at, v_t, filler):
            t0 = g * 4 * N
            w0 = w4 * N
            # -- AV + softmax-normalize, per head pair --------------------
            aoT = [win.tile([128, N], BF, tag=f"aoT{i}", name=f"aoT{i}")
                   for i in range(6)]
            jst = 512 if FLAG_AV2BANK else N  # psum column stride per head
            for p6 in range(6):
                if FLAG_AV2BANK:
                    ps = avp.tile([65, 1024], F32, tag="av", name="psav")
                else:
                    ps = avp.tile([65, 392], F32, tag="av", name="psav")
                for j in range(2):
                    s = 2 * p6 + j
                    for mci, (mo, msz) in enumerate(MC):
                        nc.tensor.matmul(
                            ps[:, j * jst:j * jst + N],
                            v_t[(w4, mci)][:msz, s * 65:(s + 1) * 65],
                            at[mci][:, s * N:(s + 1) * N],
                            start=(mci == 0) if FLAG_AV2BANK
                            else (j == 0 and mci == 0),
                            stop=(mci == 1) if FLAG_AV2BANK
                            else (j == 1 and mci == 1))
                sums_ap = (ps[64:65, :].rearrange("p (j c) -> p j c", j=2)
                           [:, :, 0:N] if FLAG_AV2BANK else ps[64:65, :])
                if FLAG_RECIP_SBUF:
                    sm = win.tile([1, 2 * N], F32, tag="sm", name="sm")
                    nc.scalar.activation(
                        sm.rearrange("p (j n) -> p j n", n=N)
                        if FLAG_AV2BANK else sm,
                        sums_ap, AF.Copy)
                    rsrc = sm
                else:
                    rsrc = sums_ap
                rr = win.tile([1, 2 * N], F32, tag="rr", name="rr")
                nc.vector.reciprocal_approx_fast(
                    rr.rearrange("p (j n) -> p j n", n=N)
                    if (FLAG_AV2BANK and not FLAG_RECIP_SBUF) else rr, rsrc)
                nrep = 128 if FLAG_RREP128 else 64
                rrep = win.tile([nrep, 2 * N], F32, tag="rrep", name="rrep")
                nc.gpsimd.partition_broadcast(rrep, rr)
                for j in range(2):
                    nc.vector.tensor_mul(
                        aoT[p6][j * 64:(j + 1) * 64, :],
                        ps[0:64, j * jst:j * jst + N],
                        rrep[0:64, j * N:(j + 1) * N])
                if p6 % 2 == 1:
                    filler(1)

            # -- projection ----------------------------------------------
            filler(2)  # cover the p6=5 normalize latency before proj needs it
            ysb = [win.tile([128, C], F32, tag=f"ysb{i}", name=f"ysb{i}")
                   for i in range(2)]
            for mci, (mo, msz) in enumerate(MC):
                pss = [mmp.tile([128, 384], F32, tag="mm", name="psp")
                       for _ in range(2)]
                for ic in range(6):
                    for half in range(2):
                        nc.tensor.matmul(
                            pss[half][:msz],
                            aoT[ic][:, mo:mo + msz],
                            wp[ic][:, half * 384:(half + 1) * 384],
                            start=(ic == 0), stop=(ic == 5))
                for half in range(2):
                    nc.scalar.copy(
                        ysb[mci][:msz, half * 384:(half + 1) * 384],
                        pss[half][:msz])
                nc.sync.dma_start(
                    out=y[t0 + w0 + mo: t0 + w0 + mo + msz, :],
                    in_=ysb[mci][:msz, :])
                filler(1)

        # Software pipeline: group g+1's q/k GEMM thunks drain as PE filler
        # inside group g's windows; v GEMM thunks of each group may spill
        # into that group's own early windows (guarded by drain_upto), which
        # also gives the last group filler work against its softmax latency.
        from collections import deque
        thunk_q = deque()
        cur_g = [0]

        def filler(k):
            for _ in range(k):
                if thunk_q:
                    thunk_q.popleft()[2]()

        def drain_upto(g, w4):
            # everything this group's window w4 needs: all earlier groups'
            # thunks, group g's qk thunks, and its v thunks for w' <= w4
            while thunk_q:
                tg, tw, fn = thunk_q[0]
                if tg < g or (tg == g and (tw < 0 or tw <= w4)):
                    thunk_q.popleft()
                    fn()
                else:
                    break

        xT0 = emit_xT(0)
        qk_c, vt_c, th0 = load_thunks(0, xT0)
        for _, _, th in th0:
            th()
        for g in range(n_grp):
            if g + 1 < n_grp:
                xTn = emit_xT(g + 1)
                qk_n, vt_n, thunks = load_thunks(g + 1, xTn)
                thunk_q.extend(thunks)
            else:
                qk_n, vt_n = None, None
            for w4 in range(4):
                drain_upto(g, w4)
                at = emit_scores(g, w4, qk_c, filler)
                emit_av_proj(g, w4, at, vt_c, filler)
            qk_c, vt_c = qk_n, vt_n
            cur_g[0] = g + 1
        while thunk_q:
            thunk_q.popleft()[2]()

    nc.compile()
    return nc


def _get_program(n_win, has_qbias):
    key = (n_win, has_qbias)
    if key not in _prog_cache:
        _prog_cache[key] = _build_program(n_win, has_qbias)
    return _prog_cache[key]


def _host_prep(x, qkv_w, q_bias, v_bias, rel_bias_table, proj_w, proj_b, H, W):
    B = x.shape[0]
    nws = H // WS  # windows per side
    xw = (np.asarray(x, np.float32)
          .reshape(B, nws, WS, nws, WS, C)
          .transpose(0, 1, 3, 2, 4, 5)
          .reshape(-1, N, C))  # [Bw, 196, C]

    scale = HD ** -0.5
    wq_s = np.array(qkv_w, np.float32, copy=True)
    wq_s[0:C] *= scale
    wqkvT = np.ascontiguousarray(wq_s.T)  # [C, 3C] f32
    # permute the v output channels into SIGMA slot order
    wqkvT[:, 2 * C:] = (wqkvT[:, 2 * C:]
                        .reshape(C, NH, HD)[:, SIGMA, :].reshape(C, C))
    wqkvT = wqkvT.astype(_BF16)

    # wp rows (attention-concat input channels) into SIGMA slot order
    wpT = np.ascontiguousarray(np.asarray(proj_w, np.float32).T)  # [C_in, C_out]
    wpT = np.ascontiguousarray(
        wpT.reshape(NH, HD, C)[SIGMA].reshape(C, C)).astype(_BF16)

    idx = _rel_index(WS).reshape(-1)
    rpb = np.asarray(rel_bias_table, np.float32)[idx].reshape(N, N, NH)  # [n,m,h]
    er_arr = np.exp(rpb).transpose(1, 2, 0)  # [m, h, n]
    er_arr = er_arr[:, SIGMA, :]             # [m, slot, n]
    er = np.ascontiguousarray(er_arr.reshape(N, NH * N)).astype(_BF16)

    qbs = np.asarray(q_bias, np.float32) * scale
    has_qbias = bool(np.any(qbs))
    qb = np.ascontiguousarray(qbs.reshape(6, 128).T).astype(_BF16)  # [128, 6]

    # v_bias and proj_b folded into a host-side output bias (exact):
    # y = (A v_raw) Wp^T + (v_bias Wp^T + proj_b)
    hb = (np.asarray(v_bias, np.float32) @ np.asarray(proj_w, np.float32).T
          + np.asarray(proj_b, np.float32))

    xbf = np.ascontiguousarray(xw.reshape(-1, C)).astype(_BF16)
    return xbf, wqkvT, wpT, er, qb, has_qbias, hb


def kernel(x, qkv_w, q_bias, v_bias, rel_bias_table, proj_w, proj_b, H, W,
           _return_results=False):
    from concourse.bass_utils import run_bass_kernel_spmd

    x = np.asarray(x)
    B = x.shape[0]
    H = int(H)
    W = int(W)
    nws = H // WS

    xbf, wqkvT, wpT, er, qb, has_qbias, hb = _host_prep(
        x, qkv_w, q_bias, v_bias, rel_bias_table, proj_w, proj_b, H, W)

    Bw = B * nws * nws
    n_win_core = Bw // NCORES
    nc = _get_program(n_win_core, has_qbias)

    tok_core = n_win_core * N
    in_maps = []
    for c in range(NCORES):
        m = {
            "x": xbf[c * tok_core:(c + 1) * tok_core],
            "wqkvT": wqkvT, "wpT": wpT, "er": er,
        }
        if has_qbias:
            m["qb"] = qb
        in_maps.append(m)

    res = run_bass_kernel_spmd(nc, in_maps, list(range(NCORES)))
    yw = np.concatenate([res.results[c]["y"] for c in range(NCORES)], axis=0)
    out = (yw.reshape(B, nws, nws, WS, WS, C)
           .transpose(0, 1, 3, 2, 4, 5)
           .reshape(B, H * W, C).astype(np.float32))
    out += hb[None, None, :]
    if _return_results:
        return out, res
    return out


# revision 33
# speedup vs baseline: 1.0749x; 1.0538x over previous
"""Swin-style windowed multi-head attention on 8 Trainium2 NeuronCores.

Problem: nn_Attention_86792699118108
  x [16, 3136, 768], 56x56 spatial, window 14x14 (no padding needed),
  12 heads, head_dim 64. 256 independent windows -> 32 windows per core.

Strategy (data-parallel over windows, v2 — dense-PE redesign):
  host: window-partition x -> bf16; pre-transpose/scale weights; permute
        heads by SIGMA so device-side attention slots are contiguous;
        pre-gather + exp the relative-position bias (exp(s+b)=exp(s)exp(b));
        fold v_bias/proj_b into a host-side output bias (exact).
  device (per core, SPMD), per group of 4 windows:
    xT (chan-major) via DMA-transpose ->
    q/k GEMM (psum 1 bank / chunk, ACT-copy eviction) ;
    v GEMM (token-major, interleaved 65-col layout with ones column) ->
    per window, per 4-head granule:
      scores: 4 matmuls into a 2-bank psum tile; head pairs at row
      groups (0,64) x banks (A,B) co-issue concurrently (PE row tiling);
      exp (ACT, one 4-head op) -> *exp(rpb) (DVE) ->
    per head-pair: AV with ones row (softmax sums free) into 1 bank;
      reciprocal (DVE, direct from psum), gpsimd partition-broadcast,
      normalize-on-eviction (DVE) ->
    proj GEMM (streams wp) -> ACT eviction -> DMA y.
    Next group's q/k/v GEMM thunks are interleaved as PE filler so the
    tensor engine never idles (keeps the HAM clock-gate at 2.4 GHz).
  host: window-reverse, + (v_bias @ proj_w.T + proj_b).
"""

import numpy as np
import ml_dtypes

WS = 14
NH = 12
HD = 64
C = 768
N = WS * WS  # 196 tokens per window
NCORES = 8

# slot -> original head; chosen so that the scores psum blocks
# [bankA0, bankA1, bankB0, bankB1] = heads [4g, 4g+2, 4g+1, 4g+3] read out
# in contiguous slot order (head pairs (2i,2i+1) co-issue into banks A/B).
SIGMA = [0, 2, 1, 3, 4, 6, 5, 7, 8, 10, 9, 11]

_BF16 = ml_dtypes.bfloat16

# debug/bisect flags (affect program structure; not part of cache key --
# set before first kernel() call only)
FLAG_EXP3D = False      # exp via two <=3D-AP ops instead of one 4D op
# reciprocal_approx_fast (custom DVE ucode) reading PSUM directly faults on
# real HW (sim accepts it) — stage the sums row through SBUF via ACT first.
FLAG_RECIP_SBUF = True
FLAG_RREP128 = False    # partition_broadcast to 128 rows (baseline shape)
FLAG_AV2BANK = False    # AV psum [65,1024], j-stride 512 (baseline layout)
FLAG_QKEVIC_GPSIMD = False  # qk psum->sbuf eviction on gpsimd instead of ACT

_prog_cache = {}


def _rel_index(ws):
    coords = np.stack(np.meshgrid(np.arange(ws), np.arange(ws), indexing="ij"))
    cf = coords.reshape(2, -1)
    rel = (cf[:, :, None] - cf[:, None, :]).transpose(1, 2, 0).astype(np.int64)
    rel[..., 0] += ws - 1
    rel[..., 1] += ws - 1
    rel[..., 0] *= 2 * ws - 1
    return rel.sum(-1)


def _build_program(n_win, has_qbias):
    import concourse.mybir as mybir
    import concourse.tile as tile
    from concourse import bacc
    from contextlib import ExitStack

    assert n_win % 4 == 0
    n_grp = n_win // 4
    n_tok = n_win * N

    BF = mybir.dt.bfloat16
    F32 = mybir.dt.float32
    AF = mybir.ActivationFunctionType

    MC = [(0, 128), (128, 68)]  # key/token chunks within a 196-token window

    nc = bacc.Bacc("TRN2", target_bir_lowering=False, debug=False,
                   num_devices=NCORES)

    x = nc.dram_tensor("x", [n_tok, C], BF, kind="ExternalInput")
    wqkvT = nc.dram_tensor("wqkvT", [C, 3 * C], BF, kind="ExternalInput")
    wpT = nc.dram_tensor("wpT", [C, C], BF, kind="ExternalInput")
    er = nc.dram_tensor("er", [N, NH * N], BF, kind="ExternalInput")
    if has_qbias:
        qb = nc.dram_tensor("qb", [128, 6], BF, kind="ExternalInput")
    y = nc.dram_tensor("y", [n_tok, C], F32, kind="ExternalOutput")

    with ExitStack() as ctx:
        tc = ctx.enter_context(tile.TileContext(nc))
        consts = ctx.enter_context(tc.tile_pool(name="consts", bufs=1))
        grp = ctx.enter_context(tc.tile_pool(name="grp", bufs=2))
        win = ctx.enter_context(tc.tile_pool(name="win", bufs=2))
        # PSUM budget: 8 banks total.
        #   scp: scores, [*,1024] f32 = 2 banks per slot, 2 slots  -> 4
        #   avp: AV,     [65,392] f32 = 1 bank per slot, 2 slots   -> 2
        #   mmp: qkv-thunk + proj psum, 1 bank per slot, 2 slots   -> 2
        scp = ctx.enter_context(tc.tile_pool(
            name="scp", bufs=1 if FLAG_AV2BANK else 2, space="PSUM"))
        avp = ctx.enter_context(tc.tile_pool(name="avp", bufs=2, space="PSUM"))
        mmp = ctx.enter_context(tc.tile_pool(name="mmp", bufs=2, space="PSUM"))

        def emit_xT(g):
            t0 = g * 4 * N
            xT = []
            for ic in range(6):
                t = grp.tile([128, 4 * N], BF, tag=f"xT{ic}", name=f"xT{ic}")
                nc.sync.dma_start(
                    out=t,
                    in_=x[t0:t0 + 4 * N, ic * 128:(ic + 1) * 128],
                    transpose=True)
                xT.append(t)
            return xT

        # ---- constants ------------------------------------------------
        # Spread across DMA queues so the first q/k thunk (which consumes
        # (wq[ic], xT[ic]) incrementally) starts ASAP: xT transposes get the
        # sync queue to themselves; wq stripes gpsimd/scalar; wp/er follow
        # (first needed ~20-40us in).
        wq = []
        for ic in range(6):
            t = consts.tile([128, 3 * C], BF, tag=f"wq{ic}", name=f"wq{ic}")
            eng = nc.scalar if ic % 2 else nc.gpsimd
            eng.dma_start(out=t, in_=wqkvT[ic * 128:(ic + 1) * 128, :])
            wq.append(t)
        xT0 = emit_xT(0)
        er_t = []
        for mci, (mo, msz) in enumerate(MC):
            t = consts.tile([msz, NH * N], BF, tag=f"er{mci}", name=f"er{mci}")
            nc.gpsimd.dma_start(out=t, in_=er[mo:mo + msz, :])
            er_t.append(t)
        wp = []
        for ic in range(6):
            t = consts.tile([128, C], BF, tag=f"wp{ic}", name=f"wp{ic}")
            nc.gpsimd.dma_start(out=t, in_=wpT[ic * 128:(ic + 1) * 128, :])
            wp.append(t)
        if has_qbias:
            qb_t = consts.tile([128, 6], BF, tag="qb", name="qb_t")
            nc.sync.dma_start(out=qb_t, in_=qb[:, :])

        def load_thunks(g, xT):
            """q/k/v GEMM for group g as fine-grained thunks (~1us PE each),
            interleaved into the previous group's attention as PE filler."""
            qk = [grp.tile([128, 4 * N], BF, tag=f"qk{oc}", name=f"qk{oc}")
                  for oc in range(12)]
            v_t = {}
            for w4 in range(4):
                for mci, (mo, msz) in enumerate(MC):
                    v_t[(w4, mci)] = grp.tile(
                        [128, NH * 65], BF, tag=f"v{w4}_{mci}",
                        name=f"v{w4}_{mci}")
            thunks = []

            def mk_qk(oc, s):
                def f():
                    ps = mmp.tile([128, 392], F32, tag="mm", name="psqk")
                    for ic in range(6):
                        nc.tensor.matmul(
                            ps,
                            wq[ic][:, oc * 128:(oc + 1) * 128],
                            xT[ic][:, s * 392:(s + 1) * 392],
                            start=(ic == 0), stop=(ic == 5))
                    if FLAG_QKEVIC_GPSIMD:
                        nc.gpsimd.tensor_copy(
                            qk[oc][:, s * 392:(s + 1) * 392], ps)
                    else:
                        nc.scalar.copy(qk[oc][:, s * 392:(s + 1) * 392], ps)
                return f

            def mk_v(w4, mci, half):
                mo, msz = MC[mci]

                def f():
                    vt = v_t[(w4, mci)]
                    vr = vt.rearrange("p (h e) -> p h e", e=65)
                    ps = mmp.tile([128, 384], F32, tag="mm", name="psv")
                    for ic in range(6):
                        nc.tensor.matmul(
                            ps[:msz],
                            xT[ic][:, w4 * N + mo: w4 * N + mo + msz],
                            wq[ic][:, 1536 + half * 384: 1536 + (half + 1) * 384],
                            start=(ic == 0), stop=(ic == 5))
                    nc.vector.tensor_copy(
                        vr[:msz, half * 6:(half + 1) * 6, 0:64],
                        ps[:msz].rearrange("p (h e) -> p h e", e=64))
                    if half == 0:
                        nc.vector.memset(vr[:msz, :, 64:65], 1.0)
                return f

            for oc in range(12):
                for s in range(2):
                    thunks.append((g, -1, mk_qk(oc, s)))
            for w4 in range(4):
                for mci in range(2):
                    for half in range(2):
                        thunks.append((g, w4, mk_v(w4, mci, half)))
            return qk, v_t, thunks

        def emit_scores(g, w4, qk, filler):
            w0 = w4 * N
            ex = []
            at = []
            for mci, (mo, msz) in enumerate(MC):
                ex.append(win.tile([msz, NH * N], BF, tag=f"ex{mci}",
                                   name=f"ex{mci}"))
                at.append(win.tile([msz, NH * N], BF, tag=f"at{mci}",
                                   name=f"at{mci}"))

            # -- scores + exp + rel-bias multiply, per 4-head granule -----
            for gr in range(3):
                for mci, (mo, msz) in enumerate(MC):
                    ps = scp.tile([128, 1024], F32, tag="sc", name="pssc")
                    # psum blocks: A0=0, A1=196 (bank A); B0=512, B1=708
                    # (bank B).  Head pairs of one qk tile go to different
                    # banks + different PE row groups -> co-issue.
                    kta, ktb = qk[6 + 2 * gr], qk[6 + 2 * gr + 1]
                    qta, qtb = qk[2 * gr], qk[2 * gr + 1]
                    nc.tensor.matmul(  # head 4gr -> A0
                        ps[:msz, 0:N],
                        kta[0:64, w0 + mo: w0 + mo + msz],
                        qta[0:64, w0:w0 + N],
                        start=True, stop=False)
                    nc.tensor.matmul(  # head 4gr+1 -> B0
                        ps[:msz, 512:512 + N],
                        kta[64:128, w0 + mo: w0 + mo + msz],
                        qta[64:128, w0:w0 + N],
                        start=True, stop=False)
                    if has_qbias:
                        # delta(m) = q_bias . k_m per head, psum cols 904+k
                        # (bank B group), exp bias operands.
                        dk = [(kta, 0, 2 * gr), (ktb, 0, 2 * gr + 1),
                              (kta, 64, 2 * gr), (ktb, 64, 2 * gr + 1)]
                        for k, (kt, ro, ti) in enumerate(dk):
                            nc.tensor.matmul(
                                ps[:msz, 904 + k:905 + k],
                                kt[ro:ro + 64, w0 + mo: w0 + mo + msz],
                                qb_t[ro:ro + 64, ti:ti + 1],
                                start=False, stop=False)
                    nc.tensor.matmul(  # head 4gr+2 -> A1
                        ps[:msz, N:2 * N],
                        ktb[0:64, w0 + mo: w0 + mo + msz],
                        qtb[0:64, w0:w0 + N],
                        start=False, stop=True)
                    nc.tensor.matmul(  # head 4gr+3 -> B1
                        ps[:msz, 512 + N:512 + 2 * N],
                        ktb[64:128, w0 + mo: w0 + mo + msz],
                        qtb[64:128, w0:w0 + N],
                        start=False, stop=True)

                    exs = (ex[mci].rearrange("p (s n) -> p s n", n=N)
                           [:, 4 * gr:4 * gr + 4, :])
                    if has_qbias:
                        for k in range(4):
                            pcol = (k // 2) * 512 + (k % 2) * N
                            nc.scalar.activation(
                                exs[:, k, :], ps[:msz, pcol:pcol + N],
                                AF.Exp, bias=ps[:msz, 904 + k:905 + k])
                    elif FLAG_EXP3D:
                        for b in range(2):
                            nc.scalar.activation(
                                exs[:, 2 * b:2 * b + 2, :],
                                ps[:msz, 512 * b:512 * b + 2 * N]
                                    .rearrange("p (k n) -> p k n", n=N),
                                AF.Exp)
                    else:
                        nc.scalar.activation(
                            exs.rearrange("p (b k) n -> p b k n", b=2),
                            ps[:msz].rearrange("p (b c) -> p b c", b=2)
                                [:, :, 0:2 * N]
                                .rearrange("p b (k n) -> p b k n", n=N),
                            AF.Exp)
                    nc.vector.tensor_mul(
                        at[mci][:, 4 * gr * N:(4 * gr + 4) * N],
                        ex[mci][:, 4 * gr * N:(4 * gr + 4) * N],
                        er_t[mci][:, 4 * gr * N:(4 * gr + 4) * N])
                filler(1)
            return at

        def emit_av_proj(g, w4, at, v_t, filler):
            t0 = g * 4 * N
            w0 = w4 * N
            # -- AV + softmax-normalize, per head pair --------------------
            aoT = [win.tile([128, N], BF, tag=f"aoT{i}", name=f"aoT{i}")
                   for i in range(6)]
            jst = 512 if FLAG_AV2BANK else N  # psum column stride per head
            for p6 in range(6):
                if FLAG_AV2BANK:
                    ps = avp.tile([65, 1024], F32, tag="av", name="psav")
                else:
                    ps = avp.tile([65, 392], F32, tag="av", name="psav")
                for j in range(2):
                    s = 2 * p6 + j
                    for mci, (mo, msz) in enumerate(MC):
                        nc.tensor.matmul(
                            ps[:, j * jst:j * jst + N],
                            v_t[(w4, mci)][:msz, s * 65:(s + 1) * 65],
                            at[mci][:, s * N:(s + 1) * N],
                            start=(mci == 0) if FLAG_AV2BANK
                            else (j == 0 and mci == 0),
                            stop=(mci == 1) if FLAG_AV2BANK
                            else (j == 1 and mci == 1))
                sums_ap = (ps[64:65, :].rearrange("p (j c) -> p j c", j=2)
                           [:, :, 0:N] if FLAG_AV2BANK else ps[64:65, :])
                if FLAG_RECIP_SBUF:
                    sm = win.tile([1, 2 * N], F32, tag="sm", name="sm")
                    nc.scalar.activation(
                        sm.rearrange("p (j n) -> p j n", n=N)
                        if FLAG_AV2BANK else sm,
                        sums_ap, AF.Copy)
                    rsrc = sm
                else:
                    rsrc = sums_ap
                rr = win.tile([1, 2 * N], F32, tag="rr", name="rr")
                nc.vector.reciprocal_approx_fast(
                    rr.rearrange("p (j n) -> p j n", n=N)
                    if (FLAG_AV2BANK and not FLAG_RECIP_SBUF) else rr, rsrc)
                nrep = 128 if FLAG_RREP128 else 64
                rrep = win.tile([nrep, 2 * N], F32, tag="rrep", name="rrep")
                nc.gpsimd.partition_broadcast(rrep, rr)
                for j in range(2):
                    nc.vector.tensor_mul(
                        aoT[p6][j * 64:(j + 1) * 64, :],
                        ps[0:64, j * jst:j * jst + N],
                        rrep[0:64, j * N:(j + 1) * N])
                if p6 % 2 == 1:
                    filler(1)

            # -- projection ----------------------------------------------
            filler(2)  # cover the p6=5 normalize latency before proj needs it
            ysb = [win.tile([128, C], F32, tag=f"ysb{i}", name=f"ysb{i}")
                   for i in range(2)]
            for mci, (mo, msz) in enumerate(MC):
                pss = [mmp.tile([128, 384], F32, tag="mm", name="psp")
                       for _ in range(2)]
                for ic in range(6):
                    for half in range(2):
                        nc.tensor.matmul(
                            pss[half][:msz],
                            aoT[ic][:, mo:mo + msz],
                            wp[ic][:, half * 384:(half + 1) * 384],
                            start=(ic == 0), stop=(ic == 5))
                for half in range(2):
                    nc.scalar.copy(
                        ysb[mci][:msz, half * 384:(half + 1) * 384],
                        pss[half][:msz])
                nc.sync.dma_start(
                    out=y[t0 + w0 + mo: t0 + w0 + mo + msz, :],
                    in_=ysb[mci][:msz, :])
                filler(1)

        # Software pipeline: group g+1's q/k GEMM thunks drain as PE filler
        # inside group g's windows; v GEMM thunks of each group may spill
        # into that group's own early windows (guarded by drain_upto), which
        # also gives the last group filler work against its softmax latency.
        from collections import deque
        thunk_q = deque()
        cur_g = [0]

        def filler(k):
            for _ in range(k):
                if thunk_q:
                    thunk_q.popleft()[2]()

        def drain_upto(g, w4):
            # everything this group's window w4 needs: all earlier groups'
            # thunks, group g's qk thunks, and its v thunks for w' <= w4
            while thunk_q:
                tg, tw, fn = thunk_q[0]
                if tg < g or (tg == g and (tw < 0 or tw <= w4)):
                    thunk_q.popleft()
                    fn()
                else:
                    break

        qk_c, vt_c, th0 = load_thunks(0, xT0)
        for _, _, th in th0:
            th()
        for g in range(n_grp):
            if g + 1 < n_grp:
                xTn = emit_xT(g + 1)
                qk_n, vt_n, thunks = load_thunks(g + 1, xTn)
                thunk_q.extend(thunks)
            else:
                qk_n, vt_n = None, None
            for w4 in range(4):
                drain_upto(g, w4)
                at = emit_scores(g, w4, qk_c, filler)
                emit_av_proj(g, w4, at, vt_c, filler)
            qk_c, vt_c = qk_n, vt_n
            cur_g[0] = g + 1
        while thunk_q:
            thunk_q.popleft()[2]()

    nc.compile()
    return nc


def _get_program(n_win, has_qbias):
    key = (n_win, has_qbias)
    if key not in _prog_cache:
        _prog_cache[key] = _build_program(n_win, has_qbias)
    return _prog_cache[key]


def _host_prep(x, qkv_w, q_bias, v_bias, rel_bias_table, proj_w, proj_b, H, W):
    B = x.shape[0]
    nws = H // WS  # windows per side
    xw = (np.asarray(x, np.float32)
          .reshape(B, nws, WS, nws, WS, C)
          .transpose(0, 1, 3, 2, 4, 5)
          .reshape(-1, N, C))  # [Bw, 196, C]

    scale = HD ** -0.5
    wq_s = np.array(qkv_w, np.float32, copy=True)
    wq_s[0:C] *= scale
    wqkvT = np.ascontiguousarray(wq_s.T)  # [C, 3C] f32
    # permute the v output channels into SIGMA slot order
    wqkvT[:, 2 * C:] = (wqkvT[:, 2 * C:]
                        .reshape(C, NH, HD)[:, SIGMA, :].reshape(C, C))
    wqkvT = wqkvT.astype(_BF16)

    # wp rows (attention-concat input channels) into SIGMA slot order
    wpT = np.ascontiguousarray(np.asarray(proj_w, np.float32).T)  # [C_in, C_out]
    wpT = np.ascontiguousarray(
        wpT.reshape(NH, HD, C)[SIGMA].reshape(C, C)).astype(_BF16)

    idx = _rel_index(WS).reshape(-1)
    rpb = np.asarray(rel_bias_table, np.float32)[idx].reshape(N, N, NH)  # [n,m,h]
    er_arr = np.exp(rpb).transpose(1, 2, 0)  # [m, h, n]
    er_arr = er_arr[:, SIGMA, :]             # [m, slot, n]
    er = np.ascontiguousarray(er_arr.reshape(N, NH * N)).astype(_BF16)

    qbs = np.asarray(q_bias, np.float32) * scale
    has_qbias = bool(np.any(qbs))
    qb = np.ascontiguousarray(qbs.reshape(6, 128).T).astype(_BF16)  # [128, 6]

    # v_bias and proj_b folded into a host-side output bias (exact):
    # y = (A v_raw) Wp^T + (v_bias Wp^T + proj_b)
    hb = (np.asarray(v_bias, np.float32) @ np.asarray(proj_w, np.float32).T
          + np.asarray(proj_b, np.float32))

    xbf = np.ascontiguousarray(xw.reshape(-1, C)).astype(_BF16)
    return xbf, wqkvT, wpT, er, qb, has_qbias, hb


def kernel(x, qkv_w, q_bias, v_bias, rel_bias_table, proj_w, proj_b, H, W,
           _return_results=False):
    from concourse.bass_utils import run_bass_kernel_spmd

    x = np.asarray(x)
    B = x.shape[0]
    H = int(H)
    W = int(W)
    nws = H // WS

    xbf, wqkvT, wpT, er, qb, has_qbias, hb = _host_prep(
        x, qkv_w, q_bias, v_bias, rel_bias_table, proj_w, proj_b, H, W)

    Bw = B * nws * nws
    n_win_core = Bw // NCORES
    nc = _get_program(n_win_core, has_qbias)

    tok_core = n_win_core * N
    in_maps = []
    for c in range(NCORES):
        m = {
            "x": xbf[c * tok_core:(c + 1) * tok_core],
            "wqkvT": wqkvT, "wpT": wpT, "er": er,
        }
        if has_qbias:
            m["qb"] = qb
        in_maps.append(m)

    res = run_bass_kernel_spmd(nc, in_maps, list(range(NCORES)))
    yw = np.concatenate([res.results[c]["y"] for c in range(NCORES)], axis=0)
    out = (yw.reshape(B, nws, nws, WS, WS, C)
           .transpose(0, 1, 3, 2, 4, 5)
           .reshape(B, H * W, C).astype(np.float32))
    out += hb[None, None, :]
    if _return_results:
        return out, res
    return out
